# revision 3
# baseline (speedup 1.0000x reference)
"""2D DWT (db2, FFT-equivalent circular conv) as TensorE matmuls on 8 trn2 cores.

Math: for each (b,c) slice X (128x128), with F[k,j] = w[t] at k=(2j+2-t) mod 128
(the circular 4-tap filter + stride-2 decimation as a 128x64 matrix):
    LL = Fl^T X Fl,  LH = Fh^T X Fl,  HL = Fl^T X Fh,  HH = Fh^T X Fh.
With W2 = [Fl | Fh] (128x128):
    stage 1:  out1 = matmul(lhsT=X,  rhs=W2) = [B_lT | B_hT]   (w on partitions)
    stage 2:  out2 = matmul(lhsT=W2, rhs=out1) = [[LL^T, LH^T], [HL^T, HH^T]]
out2 has partitions = j (W-direction output), free = i (H-direction output);
the final transpose of each 64x64 quadrant happens on the host at gather time.

Sharding: 768 (b,c) slices split contiguously, 96 per core; pure data parallel.
The per-core input shard is transposed on the host to (h, s, w) so every DMA
reads 4KB+ contiguous per partition.
"""

import numpy as np

_NCORES = 8
_S = 96          # slices per core
_G = 16          # slices per chunk (DMA/compute granularity)
_N = 128

_compiled = None


def _build_w2(w_l: np.ndarray, w_h: np.ndarray) -> np.ndarray:
    W2 = np.zeros((_N, _N), dtype=np.float32)
    for col, w in ((0, w_l), (64, w_h)):
        w = np.asarray(w, dtype=np.float32).reshape(-1)
        L = w.shape[0]
        for j in range(_N // 2):
            for t in range(L):
                W2[(2 * j + L // 2 - t) % _N, col + j] += w[t]
    return W2


def _build_nc():
    import concourse.bacc as bacc
    import concourse.tile as tile
    import concourse.mybir as mybir

    f32 = mybir.dt.float32
    nc = bacc.Bacc("TRN2", target_bir_lowering=False, debug=False)

    xs = nc.dram_tensor("xs", [_N, _S, _N], f32, kind="ExternalInput")  # (h, s, w)
    w2 = nc.dram_tensor("w2", [_N, _N], f32, kind="ExternalInput")
    out_t = nc.dram_tensor("out_t", [_N, _S, _N], f32, kind="ExternalOutput")

    n_chunks = _S // _G
    with tile.TileContext(nc) as tc:
        with (
            tc.tile_pool(name="singles", bufs=1) as singles,
            tc.tile_pool(name="xin", bufs=3) as xin,
            tc.tile_pool(name="mid", bufs=2) as mid,
            tc.tile_pool(name="out", bufs=2) as outp,
            tc.tile_pool(name="ps1", bufs=4, space="PSUM") as ps1p,
            tc.tile_pool(name="ps2", bufs=2, space="PSUM") as ps2p,
        ):
            w2_sb = singles.tile([_N, _N], f32)
            nc.sync.dma_start(out=w2_sb[:], in_=w2[:])

            for c in range(n_chunks):
                c0 = c * _G
                x_sb = xin.tile([_N, _G * _N], f32)
                nc.sync.dma_start(
                    out=x_sb[:].rearrange("p (s w) -> p s w", s=_G),
                    in_=xs[:, c0 : c0 + _G, :],
                )
                out1_sb = mid.tile([_N, _G * _N], f32)
                for s in range(_G):
                    ps1 = ps1p.tile([_N, _N], f32)
                    nc.tensor.matmul(
                        ps1[:],
                        lhsT=x_sb[:, s * _N : (s + 1) * _N],
                        rhs=w2_sb[:],
                        start=True,
                        stop=True,
                    )
                    dst = out1_sb[:, s * _N : (s + 1) * _N]
                    if s % 2 == 0:
                        nc.vector.tensor_copy(out=dst, in_=ps1[:])
                    else:
                        nc.scalar.copy(out=dst, in_=ps1[:])

                out2_sb = outp.tile([_N, _G * _N], f32)
                for g in range(_G * _N // 512):
                    ps2 = ps2p.tile([_N, 512], f32)
                    nc.tensor.matmul(
                        ps2[:],
                        lhsT=w2_sb[:],
                        rhs=out1_sb[:, g * 512 : (g + 1) * 512],
                        start=True,
                        stop=True,
                    )
                    dst = out2_sb[:, g * 512 : (g + 1) * 512]
                    if g % 2 == 0:
                        nc.vector.tensor_copy(out=dst, in_=ps2[:])
                    else:
                        nc.scalar.copy(out=dst, in_=ps2[:])

                nc.sync.dma_start(
                    out=out_t[:, c0 : c0 + _G, :],
                    in_=out2_sb[:].rearrange("p (s f) -> p s f", s=_G),
                )
    nc.finalize()
    return nc


def _get_compiled():
    global _compiled
    if _compiled is None:
        _compiled = _build_nc()
    return _compiled


def run_on_hw(x: np.ndarray, w_l: np.ndarray, w_h: np.ndarray, trace: bool = False):
    """Returns ((LL, LH, HL, HH), exec_time_ns or None)."""
    from concourse.bass_utils import run_bass_kernel_spmd

    x = np.asarray(x, dtype=np.float32)
    W2 = _build_w2(np.asarray(w_l), np.asarray(w_h))

    xf = x.reshape(-1, _N, _N)  # (768, 128, 128)
    nc = _get_compiled()
    in_maps = [
        {
            # per-core shard, transposed to (h, s, w) for contiguous DMA
            "xs": np.ascontiguousarray(xf[i * _S : (i + 1) * _S].transpose(1, 0, 2)),
            "w2": W2,
        }
        for i in range(_NCORES)
    ]
    res = run_bass_kernel_spmd(nc, in_maps, list(range(_NCORES)), trace=trace)

    quads = [[], [], [], []]  # LL, LH, HL, HH per-core chunks, each (S, 64, 64)
    for i in range(_NCORES):
        ot = res.results[i]["out_t"]  # (128, 96, 128) = [j(+64*qr), s, i(+64*qc)]
        quads[0].append(np.transpose(ot[0:64, :, 0:64], (1, 2, 0)))
        quads[1].append(np.transpose(ot[0:64, :, 64:128], (1, 2, 0)))
        quads[2].append(np.transpose(ot[64:128, :, 0:64], (1, 2, 0)))
        quads[3].append(np.transpose(ot[64:128, :, 64:128], (1, 2, 0)))

    B, C, H, W = x.shape
    out = tuple(
        np.ascontiguousarray(np.concatenate(q, axis=0)).reshape(B, C, H // 2, W // 2)
        for q in quads
    )
    return out, res.exec_time_ns


def kernel(x: np.ndarray, w_l: np.ndarray, w_h: np.ndarray):
    out, _ = run_on_hw(x, w_l, w_h, trace=False)
    return out


# revision 4
# speedup vs baseline: 1.0625x; 1.0625x over previous
"""2D DWT (db2, FFT-equivalent circular conv) as TensorE matmuls on 8 trn2 cores.

Math: for each (b,c) slice X (128x128), with F[k,j] = w[t] at k=(2j+2-t) mod 128
(the circular 4-tap filter + stride-2 decimation as a 128x64 matrix):
    LL = Fl^T X Fl,  LH = Fh^T X Fl,  HL = Fl^T X Fh,  HH = Fh^T X Fh.
With W2 = [Fl | Fh] (128x128):
    stage 1:  out1 = matmul(lhsT=X,  rhs=W2) = [B_lT | B_hT]   (w on partitions)
    stage 2:  out2 = matmul(lhsT=W2, rhs=out1) = [[LL^T, LH^T], [HL^T, HH^T]]
out2 has partitions = j (W-direction output), free = i (H-direction output);
the final transpose of each 64x64 quadrant happens on the host at gather time.

Sharding: 768 (b,c) slices split contiguously, 96 per core; pure data parallel.
The per-core input shard is transposed on the host to (h, s, w) so every DMA
reads 4KB+ contiguous per partition.
"""

import numpy as np

_NCORES = 8
_S = 96          # slices per core
_G = 16          # slices per chunk (DMA/compute granularity)
_N = 128

_compiled = None


def _build_w2(w_l: np.ndarray, w_h: np.ndarray) -> np.ndarray:
    W2 = np.zeros((_N, _N), dtype=np.float32)
    for col, w in ((0, w_l), (64, w_h)):
        w = np.asarray(w, dtype=np.float32).reshape(-1)
        L = w.shape[0]
        for j in range(_N // 2):
            for t in range(L):
                W2[(2 * j + L // 2 - t) % _N, col + j] += w[t]
    return W2


def _build_nc():
    import concourse.bacc as bacc
    import concourse.tile as tile
    import concourse.mybir as mybir

    f32 = mybir.dt.float32
    nc = bacc.Bacc("TRN2", target_bir_lowering=False, debug=False)

    xs = nc.dram_tensor("xs", [_N, _S, _N], f32, kind="ExternalInput")  # (h, s, w)
    w2 = nc.dram_tensor("w2", [_N, _N], f32, kind="ExternalInput")
    out_t = nc.dram_tensor("out_t", [_N, _S, _N], f32, kind="ExternalOutput")

    # graduated chunks: small at start (PE starts fast) and end (short tail)
    chunks = [2, 2, 4, 8, 16, 16, 16, 16, 8, 4, 2, 2]
    assert sum(chunks) == _S
    with tile.TileContext(nc) as tc:
        with (
            tc.tile_pool(name="singles", bufs=1) as singles,
            tc.tile_pool(name="xin", bufs=4) as xin,
            tc.tile_pool(name="mid", bufs=2) as mid,
            tc.tile_pool(name="out", bufs=2) as outp,
            tc.tile_pool(name="ps1", bufs=4, space="PSUM") as ps1p,
            tc.tile_pool(name="ps2", bufs=2, space="PSUM") as ps2p,
        ):
            w2_sb = singles.tile([_N, _N], f32)
            nc.sync.dma_start(out=w2_sb[:], in_=w2[:])

            nv = ns = 0  # vector/scalar copy round-robin counters
            c0 = 0
            for G in chunks:
                x_sb = xin.tile([_N, _G * _N], f32, tag="x")
                nc.sync.dma_start(
                    out=x_sb[:, : G * _N].rearrange("p (s w) -> p s w", s=G),
                    in_=xs[:, c0 : c0 + G, :],
                )
                out1_sb = mid.tile([_N, _G * _N], f32, tag="mid")
                for s in range(G):
                    ps1 = ps1p.tile([_N, _N], f32)
                    nc.tensor.matmul(
                        ps1[:],
                        lhsT=x_sb[:, s * _N : (s + 1) * _N],
                        rhs=w2_sb[:],
                        start=True,
                        stop=True,
                    )
                    dst = out1_sb[:, s * _N : (s + 1) * _N]
                    if (nv + ns) % 2 == 0:
                        nc.vector.tensor_copy(out=dst, in_=ps1[:])
                        nv += 1
                    else:
                        nc.scalar.copy(out=dst, in_=ps1[:])
                        ns += 1

                out2_sb = outp.tile([_N, _G * _N], f32, tag="out")
                ncols = G * _N
                g0 = 0
                while g0 < ncols:
                    gw = min(512, ncols - g0)
                    ps2 = ps2p.tile([_N, 512], f32)
                    nc.tensor.matmul(
                        ps2[:, :gw],
                        lhsT=w2_sb[:],
                        rhs=out1_sb[:, g0 : g0 + gw],
                        start=True,
                        stop=True,
                    )
                    dst = out2_sb[:, g0 : g0 + gw]
                    if (nv + ns) % 2 == 0:
                        nc.vector.tensor_copy(out=dst, in_=ps2[:, :gw])
                        nv += 1
                    else:
                        nc.scalar.copy(out=dst, in_=ps2[:, :gw])
                        ns += 1
                    g0 += gw

                nc.sync.dma_start(
                    out=out_t[:, c0 : c0 + G, :],
                    in_=out2_sb[:, : G * _N].rearrange("p (s f) -> p s f", s=G),
                )
                c0 += G
    nc.finalize()
    return nc


def _get_compiled():
    global _compiled
    if _compiled is None:
        _compiled = _build_nc()
    return _compiled


def run_on_hw(x: np.ndarray, w_l: np.ndarray, w_h: np.ndarray, trace: bool = False):
    """Returns ((LL, LH, HL, HH), exec_time_ns or None)."""
    from concourse.bass_utils import run_bass_kernel_spmd

    x = np.asarray(x, dtype=np.float32)
    W2 = _build_w2(np.asarray(w_l), np.asarray(w_h))

    xf = x.reshape(-1, _N, _N)  # (768, 128, 128)
    nc = _get_compiled()
    in_maps = [
        {
            # per-core shard, transposed to (h, s, w) for contiguous DMA
            "xs": np.ascontiguousarray(xf[i * _S : (i + 1) * _S].transpose(1, 0, 2)),
            "w2": W2,
        }
        for i in range(_NCORES)
    ]
    res = run_bass_kernel_spmd(nc, in_maps, list(range(_NCORES)), trace=trace)

    quads = [[], [], [], []]  # LL, LH, HL, HH per-core chunks, each (S, 64, 64)
    for i in range(_NCORES):
        ot = res.results[i]["out_t"]  # (128, 96, 128) = [j(+64*qr), s, i(+64*qc)]
        quads[0].append(np.transpose(ot[0:64, :, 0:64], (1, 2, 0)))
        quads[1].append(np.transpose(ot[0:64, :, 64:128], (1, 2, 0)))
        quads[2].append(np.transpose(ot[64:128, :, 0:64], (1, 2, 0)))
        quads[3].append(np.transpose(ot[64:128, :, 64:128], (1, 2, 0)))

    B, C, H, W = x.shape
    out = tuple(
        np.ascontiguousarray(np.concatenate(q, axis=0)).reshape(B, C, H // 2, W // 2)
        for q in quads
    )
    return out, res.exec_time_ns


def kernel(x: np.ndarray, w_l: np.ndarray, w_h: np.ndarray):
    out, _ = run_on_hw(x, w_l, w_h, trace=False)
    return out
